# revision 1
# baseline (speedup 1.0000x reference)
"""Trainium2 Bass kernel: vq_codebook (t-distribution cluster assignment).

Computes, for x (131072, 512) and cluster_centers (512, 512), all fp32:
    dist2 = relu(||x||^2 + ||c||^2 - 2 x @ c.T)
    q = 1 / (1 + dist2)            # ALPHA = 1.0 -> pow((a+1)/2) is identity
    q = q / q.sum(axis=1, keepdims=True)

Strategy (8 NeuronCores, data-parallel over rows of x):
  - Host pre-transposes each core's x shard into contract-major (d on the
    partition axis) bf16 tiles so the tensor engine needs no on-device
    transpose, and folds the affine terms into the GEMM via 4 augmented
    bf16 contraction rows: [x2_hi, x2_lo, 1, 1] . [1, 1, c2p1_hi, c2p1_lo]
    so PSUM directly holds 1 + ||x-c||^2 (hi/lo bf16 splits keep the
    large x2 (~512) and c2+1 terms accurate to ~2e-3 absolute).
  - Device: 5 accumulating matmuls per 128-row tile (4x K=128 bf16 chunks
    of -2*x.c plus the K=4 augmented chunk), then
    DVE reciprocal_approx_fast -> q, ACT Copy+accum_out -> row sums,
    DVE reciprocal -> 1/s, DVE tensor_scalar in-place scale, DMA out.
  - dist2 >= ~350 for this data (||x||^2 ~ chi2(512)), so the relu clamp
    never fires and 1+dist2 is far from reciprocal edge cases.
"""

import numpy as np
import ml_dtypes

N, D, K = 131072, 512, 512
CORES = 8
R = N // CORES            # 16384 rows per core
MROWS = 512               # rows per macro-step
MACROS = R // MROWS       # 32
T = MROWS // 128          # 4 row-tiles per macro
CH = D // 128             # 4 contraction chunks

BF16 = ml_dtypes.bfloat16

_CACHE = {}


def _build_nc(macros=MACROS):
    import concourse.bacc as bacc
    import concourse.bass as bass
    import concourse.mybir as mybir
    import concourse.tile as tile

    f32 = mybir.dt.float32
    bf16 = mybir.dt.bfloat16
    ACT_COPY = mybir.ActivationFunctionType.Copy

    rows = macros * MROWS
    nc = bacc.Bacc("TRN2", target_bir_lowering=False, debug=False)
    xt_d = nc.dram_tensor("xt", [128, macros, CH * MROWS], bf16, kind="ExternalInput").ap()
    aug_d = nc.dram_tensor("aug", [4, rows], bf16, kind="ExternalInput").ap()
    ct2_d = nc.dram_tensor("ct2", [128, CH * K], bf16, kind="ExternalInput").ap()
    augr_d = nc.dram_tensor("augr", [4, K], bf16, kind="ExternalInput").ap()
    y_d = nc.dram_tensor("y", [rows, K], f32, kind="ExternalOutput").ap()
    y_r = y_d.rearrange("(m t p) j -> m p t j", m=macros, t=T, p=128)

    with tile.TileContext(nc) as tc:
        with (
            tc.tile_pool(name="const", bufs=1) as cpool,
            tc.tile_pool(name="xin", bufs=4) as xpool,
            tc.tile_pool(name="q", bufs=2) as qpool,
            tc.tile_pool(name="out", bufs=3) as opool,
            tc.tile_pool(name="stats", bufs=4) as spool,
            tc.tile_pool(name="ps", bufs=2, space=bass.MemorySpace.PSUM) as pspool,
        ):
            ct2_sb = cpool.tile([128, CH * K], bf16)
            nc.sync.dma_start(ct2_sb[:], ct2_d[:])
            augr_sb = cpool.tile([4, K], bf16)
            nc.sync.dma_start(augr_sb[:], augr_d[:])
            aug_sb = cpool.tile([4, rows], bf16)
            nc.sync.dma_start(aug_sb[:], aug_d[:])

            for m in range(macros):
                xt_sb = xpool.tile([128, CH * MROWS], bf16)
                nc.sync.dma_start(xt_sb[:], xt_d[:, m, :])

                ps = pspool.tile([128, T * K], f32)
                for t in range(T):
                    pslice = ps[:, t * K:(t + 1) * K]
                    for k in range(CH):
                        c0 = k * MROWS + t * 128
                        nc.tensor.matmul(
                            pslice,
                            xt_sb[:, c0:c0 + 128],
                            ct2_sb[:, k * K:(k + 1) * K],
                            start=(k == 0),
                            stop=False,
                        )
                    a0 = m * MROWS + t * 128
                    nc.tensor.matmul(
                        pslice,
                        aug_sb[:, a0:a0 + 128],
                        augr_sb[:],
                        start=False,
                        stop=True,
                    )

                q_sb = qpool.tile([128, T * K], f32)
                nc.vector.reciprocal_approx_fast(q_sb[:], ps[:])

                out_sb = opool.tile([128, T * K], f32)
                s_sb = spool.tile([128, T], f32)
                for t in range(T):
                    nc.scalar.activation(
                        out_sb[:, t * K:(t + 1) * K],
                        q_sb[:, t * K:(t + 1) * K],
                        ACT_COPY,
                        accum_out=s_sb[:, t:t + 1],
                    )
                rs_sb = spool.tile([128, T], f32)
                nc.vector.reciprocal(rs_sb[:], s_sb[:])
                for t in range(T):
                    nc.vector.tensor_scalar_mul(
                        out_sb[:, t * K:(t + 1) * K],
                        out_sb[:, t * K:(t + 1) * K],
                        rs_sb[:, t:t + 1],
                    )
                nc.gpsimd.dma_start(
                    y_r[m],
                    out_sb[:].rearrange("p (t j) -> p t j", t=T),
                )

    nc.compile()
    return nc


def _bf16_hilo(v32):
    hi = v32.astype(BF16)
    lo = (v32 - hi.astype(np.float32)).astype(BF16)
    return hi, lo


def _prep_shared(cluster_centers):
    c = np.asarray(cluster_centers, np.float32)
    ct2 = (
        (-2.0 * c).T.reshape(CH, 128, K).transpose(1, 0, 2).reshape(128, CH * K)
    ).astype(BF16)
    c2p1 = (1.0 + (c.astype(np.float64) ** 2).sum(1)).astype(np.float32)
    c2p1_hi, c2p1_lo = _bf16_hilo(c2p1)
    ones = np.ones(K, BF16)
    augr = np.stack([ones, ones, c2p1_hi, c2p1_lo])
    return np.ascontiguousarray(ct2), np.ascontiguousarray(augr)


def _prep_shard(x_shard, macros=MACROS):
    xs = np.asarray(x_shard, np.float32)
    rows = macros * MROWS
    xt = (
        xs.reshape(macros, MROWS, CH, 128)
        .transpose(3, 0, 2, 1)
        .reshape(128, macros, CH * MROWS)
    ).astype(BF16)
    x2 = (xs.astype(np.float64) ** 2).sum(1).astype(np.float32)
    x2_hi, x2_lo = _bf16_hilo(x2)
    ones = np.ones(rows, BF16)
    aug = np.stack([x2_hi, x2_lo, ones, ones])
    return np.ascontiguousarray(xt), np.ascontiguousarray(aug)


def _get_nc():
    if "nc" not in _CACHE:
        _CACHE["nc"] = _build_nc()
    return _CACHE["nc"]


def make_in_maps(x, cluster_centers):
    ct2, augr = _prep_shared(cluster_centers)
    in_maps = []
    for cid in range(CORES):
        xt, aug = _prep_shard(x[cid * R:(cid + 1) * R])
        in_maps.append({"xt": xt, "aug": aug, "ct2": ct2, "augr": augr})
    return in_maps


def kernel(x, cluster_centers):
    from concourse.bass_utils import run_bass_kernel_spmd

    nc = _get_nc()
    in_maps = make_in_maps(x, cluster_centers)
    res = run_bass_kernel_spmd(nc, in_maps, list(range(CORES)))
    return np.concatenate([res.results[c]["y"] for c in range(CORES)], axis=0)


# revision 20
# speedup vs baseline: 455.1749x; 455.1749x over previous
"""Trainium2 Bass kernel: vq_codebook (t-distribution cluster assignment).

Computes, for x (131072, 512) and cluster_centers (512, 512), all fp32:
    dist2 = relu(||x||^2 + ||c||^2 - 2 x @ c.T)
    q = 1 / (1 + dist2)            # ALPHA = 1.0 -> pow((a+1)/2) is identity
    q = q / q.sum(axis=1, keepdims=True)

Strategy (8 NeuronCores, data-parallel over rows of x):
  - Host pre-transposes each core's x shard into contract-major (d on the
    partition axis) tiles so the tensor engine needs no on-device
    transpose, and folds the affine terms into the GEMM via 4 augmented
    bf16 contraction rows: [x2_hi, x2_lo, 1, 1] . [1, 1, c2p1_hi, c2p1_lo]
    so PSUM directly holds S*(1 + ||x-c||^2) (hi/lo bf16 splits keep the
    large x2 (~512) and c2+1 terms accurate; the global scale S cancels
    in the row normalization).
  - Device per 128-row tile: accumulating matmuls (bf16 K=128 chunks, or
    fp8 DoubleRow K=256 chunks, of -S*2*x.c, plus the K=4 augmented bf16
    chunk), then DVE reciprocal_approx_fast -> q, ACT Copy+accum_out ->
    row sums, DVE reciprocal -> 1/s, DVE tensor_scalar in-place scale,
    DMA out.
  - dist2 >= ~350 for this data (||x||^2 ~ chi2(512)), so the relu clamp
    never fires and the reciprocal input is far from edge cases.
"""

import numpy as np
import ml_dtypes

N, D, K = 131072, 512, 512
CORES = 8
R = N // CORES            # 16384 rows per core
MROWS = 512               # rows per macro-step
MACROS = R // MROWS       # 32
T = MROWS // 128          # 4 row-tiles per macro
CH = D // 128             # 4 contraction chunks (bf16)

BF16 = ml_dtypes.bfloat16

# Config: MAIN in {"bf16", "fp8dr"}; OUT in {"f32", "f16"}.
MAIN = "bf16"
OUT = "f32"
FP8_SCALE = 16.0

_CACHE = {}


def _np_fp8():
    import concourse.mybir as mybir
    return mybir.dt.np(mybir.dt.float8e4)


def _build_nc(macros=MACROS, reps=1, main=None, out=None, loop=1,
              out_eng="scalar", xin_bufs=6, out_bufs=4, dma_batch=1):
    import concourse.bacc as bacc
    import concourse.bass as bass
    import concourse.mybir as mybir
    import concourse.tile as tile

    main = main or MAIN
    out = out or OUT
    f32 = mybir.dt.float32
    f16 = mybir.dt.float16
    bf16 = mybir.dt.bfloat16
    fp8 = mybir.dt.float8e4
    ACT_COPY = mybir.ActivationFunctionType.Copy
    DR = mybir.MatmulPerfMode.DoubleRow

    out_dt = f32 if out == "f32" else f16
    main_dt = bf16 if main == "bf16" else fp8

    rows = macros * MROWS
    nc = bacc.Bacc("TRN2", target_bir_lowering=False, debug=False)
    xt_d = nc.dram_tensor("xt", [128, macros, CH * MROWS], main_dt, kind="ExternalInput").ap()
    aug_d = nc.dram_tensor("aug", [4, rows], bf16, kind="ExternalInput").ap()
    ct2_d = nc.dram_tensor("ct2", [128, CH * K], main_dt, kind="ExternalInput").ap()
    augr_d = nc.dram_tensor("augr", [4, K], bf16, kind="ExternalInput").ap()
    y_d = nc.dram_tensor("y", [rows, K], out_dt, kind="ExternalOutput").ap()
    y_r = y_d.rearrange("(g b t p) j -> g p b t j",
                        g=macros // dma_batch, b=dma_batch, t=T, p=128)

    with tile.TileContext(nc) as tc:
        with (
            tc.tile_pool(name="const", bufs=1) as cpool,
            tc.tile_pool(name="xin", bufs=xin_bufs) as xpool,
            tc.tile_pool(name="q", bufs=2) as qpool,
            tc.tile_pool(name="out", bufs=out_bufs) as opool,
            tc.tile_pool(name="stats", bufs=4) as spool,
            tc.tile_pool(name="ps", bufs=2, space=bass.MemorySpace.PSUM) as pspool,
        ):
            ct2_sb = cpool.tile([128, CH * K], main_dt)
            nc.sync.dma_start(ct2_sb[:], ct2_d[:])
            augr_sb = cpool.tile([4, K], bf16)
            nc.sync.dma_start(augr_sb[:], augr_d[:])
            aug_sb = cpool.tile([4, rows], bf16)
            nc.sync.dma_start(aug_sb[:], aug_d[:])

            import contextlib
            loop_cm = tc.For_i(0, loop, 1) if loop > 1 else contextlib.nullcontext()
            with loop_cm:
                body(nc, tc, reps, macros, main, xt_d, y_r, xt_sb_pool=xpool,
                     qpool=qpool, opool=opool, spool=spool, pspool=pspool,
                     ct2_sb=ct2_sb, augr_sb=augr_sb, aug_sb=aug_sb,
                     main_dt=main_dt, out_dt=out_dt, f32=f32,
                     ACT_COPY=ACT_COPY, DR=DR, out_eng=out_eng,
                     dma_batch=dma_batch)

    nc.compile()
    return nc


def body(nc, tc, reps, macros, main, xt_d, y_r, xt_sb_pool, qpool, opool,
         spool, pspool, ct2_sb, augr_sb, aug_sb, main_dt, out_dt, f32,
         ACT_COPY, DR, out_eng="gpsimd", dma_batch=1):
    xpool = xt_sb_pool
    out_dma = getattr(nc, out_eng)
    B = dma_batch
    if True:
            for m in [mm for _ in range(reps) for mm in range(macros)]:
                if m % B == 0:
                    xt_sbb = xpool.tile([128, B * CH * MROWS], main_dt)
                    nc.sync.dma_start(
                        xt_sbb[:].rearrange("p (b c) -> p b c", b=B),
                        xt_d[:, m:m + B, :],
                    )
                    out_sbb = opool.tile([128, B * T * K], out_dt)
                xt_sb = xt_sbb[:, (m % B) * CH * MROWS:(m % B + 1) * CH * MROWS]

                ps = pspool.tile([128, T * K], f32)
                for t in range(T):
                    pslice = ps[:, t * K:(t + 1) * K]
                    if main == "bf16":
                        for k in range(CH):
                            c0 = k * MROWS + t * 128
                            nc.tensor.matmul(
                                pslice,
                                xt_sb[:, c0:c0 + 128],
                                ct2_sb[:, k * K:(k + 1) * K],
                                start=(k == 0),
                                stop=False,
                            )
                    else:
                        for k2 in range(2):
                            a0 = k2 * 1024 + t * 256
                            lhs3 = xt_sb[:, a0:a0 + 256].rearrange(
                                "p (i v) -> p i v", i=2)
                            rhs3 = ct2_sb[:, k2 * 1024:(k2 + 1) * 1024].rearrange(
                                "p (i j) -> p i j", i=2)
                            nc.tensor.matmul(
                                pslice, lhs3, rhs3,
                                start=(k2 == 0), stop=False, perf_mode=DR,
                            )
                    a0 = m * MROWS + t * 128
                    nc.tensor.matmul(
                        pslice,
                        aug_sb[:, a0:a0 + 128],
                        augr_sb[:],
                        start=False,
                        stop=True,
                    )

                q_sb = qpool.tile([128, T * K], f32)
                nc.vector.reciprocal_approx_fast(q_sb[:], ps[:])

                out_sb = out_sbb[:, (m % B) * T * K:(m % B + 1) * T * K]
                s_sb = spool.tile([128, T], f32)
                for t in range(T):
                    nc.scalar.activation(
                        out_sb[:, t * K:(t + 1) * K],
                        q_sb[:, t * K:(t + 1) * K],
                        ACT_COPY,
                        accum_out=s_sb[:, t:t + 1],
                    )
                rs_sb = spool.tile([128, T], f32)
                nc.vector.reciprocal(rs_sb[:], s_sb[:])
                for t in range(T):
                    nc.vector.tensor_scalar_mul(
                        out_sb[:, t * K:(t + 1) * K],
                        out_sb[:, t * K:(t + 1) * K],
                        rs_sb[:, t:t + 1],
                    )
                if m % B == B - 1:
                    out_dma.dma_start(
                        y_r[m // B],
                        out_sbb[:].rearrange("p (b t j) -> p b t j", b=B, t=T),
                    )


def _bf16_hilo(v32):
    hi = v32.astype(BF16)
    lo = (v32 - hi.astype(np.float32)).astype(BF16)
    return hi, lo


def _prep_shared(cluster_centers, main=None):
    main = main or MAIN
    c = np.asarray(cluster_centers, np.float32)
    scale = 1.0 if main == "bf16" else FP8_SCALE
    w = (-2.0 * scale) * c
    if main == "bf16":
        ct2 = (
            w.T.reshape(CH, 128, K).transpose(1, 0, 2).reshape(128, CH * K)
        ).astype(BF16)
    else:
        ct2 = (
            w.T.reshape(2, 2, 128, K).transpose(2, 0, 1, 3).reshape(128, CH * K)
        ).astype(_np_fp8())
    c2p1 = ((1.0 + (c.astype(np.float64) ** 2).sum(1)) * scale).astype(np.float32)
    c2p1_hi, c2p1_lo = _bf16_hilo(c2p1)
    ones = np.ones(K, BF16)
    augr = np.stack([ones, ones, c2p1_hi, c2p1_lo])
    return np.ascontiguousarray(ct2), np.ascontiguousarray(augr)


def _prep_shard(x_shard, macros=MACROS, main=None):
    main = main or MAIN
    xs = np.asarray(x_shard, np.float32)
    rows = macros * MROWS
    scale = 1.0 if main == "bf16" else FP8_SCALE
    if main == "bf16":
        xt = (
            xs.reshape(macros, MROWS, CH, 128)
            .transpose(3, 0, 2, 1)
            .reshape(128, macros, CH * MROWS)
        ).astype(BF16)
    else:
        xt = (
            xs.reshape(macros, T, 128, 2, 2, 128)
            .transpose(5, 0, 3, 1, 4, 2)
            .reshape(128, macros, CH * MROWS)
        ).astype(_np_fp8())
    x2 = ((xs.astype(np.float64) ** 2).sum(1) * scale).astype(np.float32)
    x2_hi, x2_lo = _bf16_hilo(x2)
    ones = np.ones(rows, BF16)
    aug = np.stack([x2_hi, x2_lo, ones, ones])
    return np.ascontiguousarray(xt), np.ascontiguousarray(aug)


def _get_nc():
    if "nc" not in _CACHE:
        _CACHE["nc"] = _build_nc()
    return _CACHE["nc"]


def make_in_maps(x, cluster_centers, main=None):
    ct2, augr = _prep_shared(cluster_centers, main=main)
    in_maps = []
    for cid in range(CORES):
        xt, aug = _prep_shard(x[cid * R:(cid + 1) * R], main=main)
        in_maps.append({"xt": xt, "aug": aug, "ct2": ct2, "augr": augr})
    return in_maps


def kernel(x, cluster_centers):
    from concourse.bass_utils import run_bass_kernel_spmd

    nc = _get_nc()
    in_maps = make_in_maps(x, cluster_centers)
    res = run_bass_kernel_spmd(nc, in_maps, list(range(CORES)))
    y = np.concatenate([res.results[c]["y"] for c in range(CORES)], axis=0)
    return np.ascontiguousarray(y.astype(np.float32))
